# revision 17
# baseline (speedup 1.0000x reference)
"""CAM-module kernel for Trainium2, data-parallel over batch on 8 NeuronCores.

Per core (one batch sample, q = x[b] viewed as (C=512, N=4096) fp32):
  energy   = q @ q^T                      (C, C)   fp8 DoubleRow matmul, fp32 accum
  att[c,d] = softmax(max_d(energy) - energy)[c,d]
           = exp(m_c - e[c,d]) / Z_c      with m_c = row min of energy
  out      = gamma * (att @ q) + x

The row-max shift of the reference softmax cancels algebraically; only the
row minimum is needed for numerical stability (arguments of exp stay <= 0,
and Z >= 1 always since the min element contributes exp(0)).

Pipeline structure (phases bounded by the 23.4us HBM floor in each direction):
  A: x streams in (HBM-saturated); casts split across ACT+DVE; PE transposes
     128x128 fp8 blocks into contiguous PSUM banks that DVE copies out in
     2KB groups via u16-bitcast (2x mode); DoubleRow energy matmuls accumulate
     the upper-triangle blocks per 256-wide k-pair.
  B: lower-triangle mirrored from the symmetric upper blocks; per-ci softmax
     (exp with accum_out row sum; attention left unnormalized in fp8).
  C: att@q DoubleRow matmuls; one DVE scalar_tensor_tensor fuses the
     (gamma/Z) row scale and the +x add; output streams out (HBM-saturated).
"""

import numpy as np

import concourse.bass as bass
import concourse.tile as tile
from concourse import mybir
from concourse.masks import make_identity
from concourse.vector_clock import ScopedClock

P = 128
C = 512
N = 4096
B = 8
CT = C // P   # 4 c-tiles
KT = N // P   # 32 n-chunks of 128
NG = KT // 4  # 8 transpose groups of 4 n-chunks

STRIP_TAIL = True

FP32 = mybir.dt.float32
FP8 = mybir.dt.float8e4
U16 = mybir.dt.uint16
DR = mybir.MatmulPerfMode.DoubleRow
ALU = mybir.AluOpType


def _drain_and_barrier_split(self, tick_clock, wait_clock):
    # The pinned walrus rejects >1 sync-wait on TPB_CTRL (Drain); spread the
    # final global-clock waits across a chain of drains, one wait each.
    nc = self.nc
    drain_inst = nc.sync.drain()
    wait_clock.add_sem_waits(
        drain_inst.ins, ScopedClock({None: tick_clock.global_clock})
    )
    si = drain_inst.ins.sync_info
    if si is not None and si.on_wait is not None and len(si.on_wait) > 1:
        # Distribute the final global-clock waits round-robin across engine
        # drain chains (one wait per drain) so they resolve in parallel
        # instead of serializing ~12 drains on the Sync queue.
        waits = list(si.on_wait)
        si.on_wait = waits[:1]
        engines = [nc.sync, nc.tensor, nc.vector, nc.scalar, nc.gpsimd]
        for i, w in enumerate(waits[1:]):
            eng = engines[i % len(engines)]
            extra = eng.drain()
            extra.ins.sync_info = mybir.SyncInfo(on_wait=[w], on_update=[])
    nc.all_engine_barrier()
    assert self.sems is not None
    popped = nc._tile_sem_poison_stack.pop()
    assert popped is self._sem_poison
    if not STRIP_TAIL:
        nc.clear_and_free_semaphores(list(self.sems.allocated().values()))
        nc.all_engine_barrier()


tile.TileContext._drain_and_barrier = _drain_and_barrier_split


def _legalize_sync_waits(nc):
    # This walrus build rejects instructions carrying more than one sync-wait.
    # Hoist extra waits onto same-engine NoOps placed immediately before the
    # instruction (engine streams preserve relative order within a block).
    for f in nc.m.functions:
        for bb in f.blocks:
            new = []
            for inst in bb.instructions:
                si = inst.sync_info
                if si is not None and si.on_wait and len(si.on_wait) > 1:
                    waits = list(si.on_wait)
                    for w in waits[:-1]:
                        nop = mybir.InstNoOp(
                            name=nc.get_next_instruction_name(),
                            engine=inst.engine,
                            bass_nofuse=True,
                            sync_info=mybir.SyncInfo(on_wait=[w], on_update=[]),
                        )
                        new.append(nop)
                    si.on_wait = [waits[-1]]
                new.append(inst)
            bb.instructions[:] = new


def build_nc(prologue=True):
    nc = bass.Bass()
    x_d = nc.declare_dram_parameter("x", [C, N], FP32, isOutput=False)
    g_d = nc.declare_dram_parameter("gamma", [1, 1], FP32, isOutput=False)
    o_d = nc.declare_dram_parameter("out", [C, N], FP32, isOutput=True)

    # Clear kernel semaphores at START (idle window) instead of paying the
    # expensive teardown clear+barrier at the end (STRIP_TAIL above).
    # (Direct CoreSim can't execute the NRT pseudo barrier; prologue=False.)
    if prologue:
        from concourse.bass import compact_to_ranges

        for sem_range in compact_to_ranges(
            [sem for sem in nc._kernel_sem_range if sem not in nc.barrier_sems]
        ):
            nc.gpsimd.dma_reset(sem_range)
            nc.gpsimd.sem_clear(sem_range)
        nc._nrt_pseudo_barrier()

    with tile.TileContext(nc) as tc:
        with (
            tc.tile_pool(name="singles", bufs=1) as singles,
            tc.tile_pool(name="stage", bufs=4) as stage,
            tc.tile_pool(name="mst", bufs=2) as mst,
            tc.tile_pool(name="psum_e", bufs=4, space="PSUM") as psum_e,
            tc.tile_pool(name="psum_t", bufs=2, space="PSUM") as psum_t,
            tc.tile_pool(name="psum_o", bufs=2, space="PSUM") as psum_o,
        ):
            # ---- x load DMAs issued first so HBM streams from the earliest
            # possible moment; everything below overlaps under their shadow.
            xf = [
                singles.tile([P, N], FP32, tag=f"xf{ci}", name=f"xf{ci}")
                for ci in range(CT)
            ]
            for gg in range(4):
                base = gg * 1024
                for ci in range(CT):
                    rows = slice(ci * P, (ci + 1) * P)
                    if gg in (0, 3):
                        # finer loads at the edges: earlier pipeline light-up
                        # (gg0) and a shorter post-stream tail (gg3)
                        for c0, c1 in ((0, 512), (512, 1024)):
                            nc.sync.dma_start(
                                out=xf[ci][:, base + c0 : base + c1],
                                in_=x_d[rows, base + c0 : base + c1],
                            )
                    else:
                        nc.sync.dma_start(
                            out=xf[ci][:, base : base + 1024],
                            in_=x_d[rows, base : base + 1024],
                        )

            # PE warm-up on a dep-free tile (HAM un-throttle + pipeline),
            # ACT Exp-table preload on a dummy, identities, gamma broadcast.
            warm8 = singles.tile([P, P], FP8, tag="warm8")
            nc.vector.memset(warm8[:], 1.0)
            for _ in range(56):
                wp = psum_o.tile([P, C], FP32, tag="po")
                nc.tensor.matmul(
                    wp[:, 0:P], lhsT=warm8[:], rhs=warm8[:], start=True, stop=True
                )
            dume = singles.tile([P, 1], FP32, tag="dume")
            nc.scalar.activation(
                out=dume[:], in_=warm8[:, 0:1], func=mybir.ActivationFunctionType.Exp
            )
            id8 = singles.tile([P, P], FP8, tag="id8")
            make_identity(nc, id8)
            id32 = singles.tile([P, P], FP32, tag="id32")
            make_identity(nc, id32)
            gcol = singles.tile([P, 1], FP32, tag="gamma")
            nc.gpsimd.dma_start(out=gcol[:], in_=g_d[:, :].to_broadcast((P, 1)))

            q8 = singles.tile([P, CT, N], FP8, tag="q8")
            qT = singles.tile([P, KT, C], FP8, tag="qT")
            e_ps = [
                psum_e.tile([P, C], FP32, tag="acc", name=f"e{ci}")
                for ci in range(CT)
            ]

            # ---- Phase A: cast (ACT/DVE/GPSIMD split), transpose 128x128 fp8
            # blocks into stride-2 PSUM (HW: "FP8 transpose mode must have
            # output element step of 2"), one strided copy per k-pair split
            # ACT/DVE, DoubleRow energy matmuls per 256-wide k-pair.
            def cast(ci, lo, hi, eng):
                src = xf[ci][:, lo:hi]
                dst = q8[:, ci, lo:hi]
                if eng == "v":
                    nc.vector.tensor_copy(out=dst, in_=src)
                elif eng == "g":
                    nc.gpsimd.tensor_copy(out=dst, in_=src)
                else:
                    nc.scalar.copy(out=dst, in_=src)

            # Phase A with software-pipelined PE stream: the energy matmuls
            # for k-pair t are emitted after the transposes of t+1, so the PE
            # never stalls on the (DVE) qT copy of its own group.
            def emit_transposes(t, split_copy=False):
                pt = psum_t.tile([P, 2, C, 2], FP8, tag="tr", name="pt")
                for kk in range(2):
                    k = 2 * t + kk
                    for ci in range(CT):
                        nc.tensor.transpose(
                            pt[:, kk, ci * P : (ci + 1) * P, 0],
                            q8[:, ci, k * P : (k + 1) * P],
                            id8[:],
                        )
                if split_copy:
                    nc.scalar.copy(
                        out=qT[:, 2 * t, :], in_=pt[:, 0, :, 0]
                    )
                    nc.vector.tensor_copy(
                        out=qT[:, 2 * t + 1, :], in_=pt[:, 1, :, 0]
                    )
                else:
                    nc.vector.tensor_copy(
                        out=qT[:, 2 * t : 2 * t + 2, :], in_=pt[:, :, :, 0]
                    )

            def emit_energy(t):
                for ci in range(CT):
                    nc.tensor.matmul(
                        e_ps[ci][:, ci * P :],
                        lhsT=qT[:, 2 * t : 2 * t + 2, ci * P : (ci + 1) * P],
                        rhs=qT[:, 2 * t : 2 * t + 2, ci * P :],
                        start=(t == 0),
                        stop=(t == KT // 2 - 1),
                        perf_mode=DR,
                    )

            for gg in range(4):
                base = gg * 1024
                if gg in (0, 3):
                    for ci in range(CT):
                        cast(ci, base, base + 512, "s" if ci < 2 else "v")
                    if gg == 3:
                        for tt in range(2):
                            t = gg * 4 + tt
                            emit_transposes(t, split_copy=True)
                            emit_energy(t - 1)
                    for ci in range(CT):
                        cast(ci, base + 512, base + 1024, "s" if ci < 2 else "v")
                    for tt in range((2 if gg == 3 else 0), 4):
                        t = gg * 4 + tt
                        emit_transposes(t, split_copy=(gg == 3))
                        if t > 0:
                            emit_energy(t - 1)
                else:
                    for ci in range(CT):
                        cast(ci, base, base + 1024, "s")
                    for tt in range(4):
                        t = gg * 4 + tt
                        emit_transposes(t)
                        if t > 0:
                            emit_energy(t - 1)
            emit_energy(KT // 2 - 1)

            # ---- Phases B+C, pipelined per ci: mirror the lower-triangle
            # energy blocks this row needs (from the symmetric upper ones),
            # softmax, transpose the attention rows, att@q, fused
            # row-scale + x add, stream out.
            EXPQ = singles.tile([P, CT, C], FP8, tag="EXPQ")
            EXPT = singles.tile([P, CT, C], FP8, tag="EXPT")
            mcol = [singles.tile([P, 1], FP32, tag=f"m{ci}", name=f"m{ci}") for ci in range(CT)]
            zcol = [singles.tile([P, 1], FP32, tag=f"z{ci}", name=f"z{ci}") for ci in range(CT)]
            bias2 = [singles.tile([P, 1], FP32, tag=f"b{ci}", name=f"b{ci}") for ci in range(CT)]

            # All of phase B is emitted before any phase-C STT so the DVE
            # queue never buries a later row-block's min/mirror behind a
            # batch of output tiles (in-order engine queues).
            for ci in range(CT):
                # mirrors feeding row-block ci (none for ci == 0)
                for dj in range(ci):
                    low = mst.tile([P, P], FP32, tag="low", name="low")
                    nc.vector.tensor_copy(
                        out=low[:], in_=e_ps[dj][:, ci * P : (ci + 1) * P]
                    )
                    nc.tensor.transpose(
                        e_ps[ci][:, dj * P : (dj + 1) * P], low[:], id32[:]
                    )
                nc.vector.tensor_reduce(
                    out=mcol[ci][:],
                    in_=e_ps[ci][:],
                    axis=mybir.AxisListType.X,
                    op=ALU.min,
                )
                nc.scalar.activation(
                    out=EXPQ[:, ci, :],
                    in_=e_ps[ci][:],
                    func=mybir.ActivationFunctionType.Exp,
                    bias=mcol[ci][:],
                    scale=-1.0,
                    accum_out=zcol[ci][:],
                )
                # Z >= 1 always (min element contributes exp(0)), no clamp.
                nc.vector.reciprocal(out=bias2[ci][:], in_=zcol[ci][:])
                nc.vector.tensor_mul(
                    out=bias2[ci][:], in0=bias2[ci][:], in1=gcol[:]
                )
                ptx = psum_t.tile([P, 2, C, 2], FP8, tag="tr", name="ptx")
                for dj in range(CT):
                    nc.tensor.transpose(
                        ptx[:, dj // 2, (dj % 2) * P : (dj % 2 + 1) * P, 0],
                        EXPQ[:, ci, dj * P : (dj + 1) * P],
                        id8[:],
                    )
                for a in range(2):
                    nc.scalar.copy(
                        out=EXPT[:, 2 * a : 2 * a + 2, ci * P : (ci + 1) * P],
                        in_=ptx[:, a, 0 : 2 * P, 0],
                    )

            # ---- Phase C: att@q, fused row-scale + x add, stream out.
            # Row-blocks 2-3 borrow the freed energy banks for a deeper
            # po pipeline.
            for ci in range(CT):
                for nh in range(2):
                    osb = stage.tile([P, 2048], FP32, tag="osb", name="osb")
                    for sub in range(4):
                        nj = nh * 4 + sub
                        pool = psum_o if ci < 2 else psum_e
                        po = pool.tile(
                            [P, C], FP32,
                            tag=("po" if ci < 2 else "acc"),
                            name="po",
                        )
                        for j in range(2):
                            nc.tensor.matmul(
                                po[:],
                                lhsT=EXPT[:, 2 * j : 2 * j + 2, ci * P : (ci + 1) * P],
                                rhs=q8[:, 2 * j : 2 * j + 2, nj * C : (nj + 1) * C],
                                start=(j == 0),
                                stop=(j == 1),
                                perf_mode=DR,
                            )
                        nc.vector.scalar_tensor_tensor(
                            out=osb[:, sub * C : (sub + 1) * C],
                            in0=po[:],
                            scalar=bias2[ci][:],
                            in1=xf[ci][:, nj * C : (nj + 1) * C],
                            op0=ALU.mult,
                            op1=ALU.add,
                        )
                        if sub % 2 == 1:
                            nc.sync.dma_start(
                                out=o_d[
                                    ci * P : (ci + 1) * P,
                                    (nj - 1) * C : (nj + 1) * C,
                                ],
                                in_=osb[:, (sub - 1) * C : (sub + 1) * C],
                            )
    _legalize_sync_waits(nc)
    return nc


def make_in_maps(x, gamma):
    x = np.ascontiguousarray(np.asarray(x, dtype=np.float32)).reshape(B, C, N)
    g = np.ascontiguousarray(np.asarray(gamma, dtype=np.float32)).reshape(1, 1)
    return [{"x": x[i], "gamma": g} for i in range(B)]


def kernel(x, y=None, gamma=None, **_ignored):
    from concourse.bass_utils import run_bass_kernel_spmd

    nc = build_nc()
    in_maps = make_in_maps(x, gamma)
    res = run_bass_kernel_spmd(nc, in_maps, list(range(B)))
    out = np.stack([np.asarray(res.results[i]["out"]) for i in range(B)])
    return out.reshape(B, C, 64, 64).astype(np.float32)


# revision 18
# speedup vs baseline: 1.0290x; 1.0290x over previous
"""CAM-module kernel for Trainium2, data-parallel over batch on 8 NeuronCores.

Per core (one batch sample, q = x[b] viewed as (C=512, N=4096) fp32):
  energy   = q @ q^T                      (C, C)   fp8 DoubleRow matmul, fp32 accum
  att[c,d] = softmax(max_d(energy) - energy)[c,d]
           = exp(m_c - e[c,d]) / Z_c      with m_c = row min of energy
  out      = gamma * (att @ q) + x

The row-max shift of the reference softmax cancels algebraically; only the
row minimum is needed for numerical stability (arguments of exp stay <= 0,
and Z >= 1 always since the min element contributes exp(0)).

Pipeline structure (phases bounded by the 23.4us HBM floor in each direction):
  A: x streams in (HBM-saturated); casts split across ACT+DVE; PE transposes
     128x128 fp8 blocks into contiguous PSUM banks that DVE copies out in
     2KB groups via u16-bitcast (2x mode); DoubleRow energy matmuls accumulate
     the upper-triangle blocks per 256-wide k-pair.
  B: lower-triangle mirrored from the symmetric upper blocks; per-ci softmax
     (exp with accum_out row sum; attention left unnormalized in fp8).
  C: att@q DoubleRow matmuls; one DVE scalar_tensor_tensor fuses the
     (gamma/Z) row scale and the +x add; output streams out (HBM-saturated).
"""

import numpy as np

import concourse.bass as bass
import concourse.tile as tile
from concourse import mybir
from concourse.masks import make_identity
from concourse.vector_clock import ScopedClock

P = 128
C = 512
N = 4096
B = 8
CT = C // P   # 4 c-tiles
KT = N // P   # 32 n-chunks of 128
NG = KT // 4  # 8 transpose groups of 4 n-chunks

STRIP_TAIL = True

FP32 = mybir.dt.float32
FP8 = mybir.dt.float8e4
U16 = mybir.dt.uint16
DR = mybir.MatmulPerfMode.DoubleRow
ALU = mybir.AluOpType


def _drain_and_barrier_split(self, tick_clock, wait_clock):
    # The pinned walrus rejects >1 sync-wait on TPB_CTRL (Drain); spread the
    # final global-clock waits across a chain of drains, one wait each.
    nc = self.nc
    drain_inst = nc.sync.drain()
    wait_clock.add_sem_waits(
        drain_inst.ins, ScopedClock({None: tick_clock.global_clock})
    )
    si = drain_inst.ins.sync_info
    if si is not None and si.on_wait is not None and len(si.on_wait) > 1:
        # Distribute the final global-clock waits round-robin across engine
        # drain chains (one wait per drain) so they resolve in parallel
        # instead of serializing ~12 drains on the Sync queue.
        waits = list(si.on_wait)
        si.on_wait = waits[:1]
        engines = [nc.sync, nc.tensor, nc.vector, nc.scalar, nc.gpsimd]
        for i, w in enumerate(waits[1:]):
            eng = engines[i % len(engines)]
            extra = eng.drain()
            extra.ins.sync_info = mybir.SyncInfo(on_wait=[w], on_update=[])
    nc.all_engine_barrier()
    assert self.sems is not None
    popped = nc._tile_sem_poison_stack.pop()
    assert popped is self._sem_poison
    if not STRIP_TAIL:
        nc.clear_and_free_semaphores(list(self.sems.allocated().values()))
        nc.all_engine_barrier()


tile.TileContext._drain_and_barrier = _drain_and_barrier_split


def _legalize_sync_waits(nc):
    # This walrus build rejects instructions carrying more than one sync-wait.
    # Hoist extra waits onto same-engine NoOps placed immediately before the
    # instruction (engine streams preserve relative order within a block).
    for f in nc.m.functions:
        for bb in f.blocks:
            new = []
            for inst in bb.instructions:
                si = inst.sync_info
                if si is not None and si.on_wait and len(si.on_wait) > 1:
                    waits = list(si.on_wait)
                    for w in waits[:-1]:
                        nop = mybir.InstNoOp(
                            name=nc.get_next_instruction_name(),
                            engine=inst.engine,
                            bass_nofuse=True,
                            sync_info=mybir.SyncInfo(on_wait=[w], on_update=[]),
                        )
                        new.append(nop)
                    si.on_wait = [waits[-1]]
                new.append(inst)
            bb.instructions[:] = new


def build_nc(prologue=True):
    nc = bass.Bass()
    x_d = nc.declare_dram_parameter("x", [C, N], FP32, isOutput=False)
    g_d = nc.declare_dram_parameter("gamma", [1, 1], FP32, isOutput=False)
    o_d = nc.declare_dram_parameter("out", [C, N], FP32, isOutput=True)

    # Clear kernel semaphores at START (idle window) instead of paying the
    # expensive teardown clear+barrier at the end (STRIP_TAIL above).
    # (Direct CoreSim can't execute the NRT pseudo barrier; prologue=False.)
    if prologue:
        from concourse.bass import compact_to_ranges

        for sem_range in compact_to_ranges(
            [sem for sem in nc._kernel_sem_range if sem not in nc.barrier_sems]
        ):
            nc.gpsimd.dma_reset(sem_range)
            nc.gpsimd.sem_clear(sem_range)
        nc._nrt_pseudo_barrier()

    with tile.TileContext(nc) as tc:
        with (
            tc.tile_pool(name="singles", bufs=1) as singles,
            tc.tile_pool(name="stage", bufs=4) as stage,
            tc.tile_pool(name="mst", bufs=2) as mst,
            tc.tile_pool(name="psum_e", bufs=4, space="PSUM") as psum_e,
            tc.tile_pool(name="psum_t", bufs=2, space="PSUM") as psum_t,
            tc.tile_pool(name="psum_o", bufs=2, space="PSUM") as psum_o,
        ):
            # ---- x load DMAs issued first so HBM streams from the earliest
            # possible moment; everything below overlaps under their shadow.
            xf = [
                singles.tile([P, N], FP32, tag=f"xf{ci}", name=f"xf{ci}")
                for ci in range(CT)
            ]
            for gg in range(4):
                base = gg * 1024
                for ci in range(CT):
                    rows = slice(ci * P, (ci + 1) * P)
                    if gg in (0, 3):
                        # finer loads at the edges: earlier pipeline light-up
                        # (gg0) and a shorter post-stream tail (gg3)
                        for c0, c1 in ((0, 512), (512, 1024)):
                            nc.sync.dma_start(
                                out=xf[ci][:, base + c0 : base + c1],
                                in_=x_d[rows, base + c0 : base + c1],
                            )
                    else:
                        nc.sync.dma_start(
                            out=xf[ci][:, base : base + 1024],
                            in_=x_d[rows, base : base + 1024],
                        )

            # PE warm-up on a dep-free tile (HAM un-throttle + pipeline),
            # ACT Exp-table preload on a dummy, identities, gamma broadcast.
            warm8 = singles.tile([P, P], FP8, tag="warm8")
            nc.vector.memset(warm8[:], 1.0)
            for _ in range(24):
                wp = psum_o.tile([P, C], FP32, tag="po")
                nc.tensor.matmul(
                    wp[:, 0:P], lhsT=warm8[:], rhs=warm8[:], start=True, stop=True
                )
            dume = singles.tile([P, 1], FP32, tag="dume")
            nc.scalar.activation(
                out=dume[:], in_=warm8[:, 0:1], func=mybir.ActivationFunctionType.Exp
            )
            id8 = singles.tile([P, P], FP8, tag="id8")
            make_identity(nc, id8)
            id32 = singles.tile([P, P], FP32, tag="id32")
            make_identity(nc, id32)
            gcol = singles.tile([P, 1], FP32, tag="gamma")
            nc.gpsimd.dma_start(out=gcol[:], in_=g_d[:, :].to_broadcast((P, 1)))

            q8 = singles.tile([P, CT, N], FP8, tag="q8")
            qT = singles.tile([P, KT, C], FP8, tag="qT")
            e_ps = [
                psum_e.tile([P, C], FP32, tag="acc", name=f"e{ci}")
                for ci in range(CT)
            ]

            # ---- Phase A: cast (ACT/DVE/GPSIMD split), transpose 128x128 fp8
            # blocks into stride-2 PSUM (HW: "FP8 transpose mode must have
            # output element step of 2"), one strided copy per k-pair split
            # ACT/DVE, DoubleRow energy matmuls per 256-wide k-pair.
            def cast(ci, lo, hi, eng):
                src = xf[ci][:, lo:hi]
                dst = q8[:, ci, lo:hi]
                if eng == "v":
                    nc.vector.tensor_copy(out=dst, in_=src)
                elif eng == "g":
                    nc.gpsimd.tensor_copy(out=dst, in_=src)
                else:
                    nc.scalar.copy(out=dst, in_=src)

            # Phase A with software-pipelined PE stream: the energy matmuls
            # for k-pair t are emitted after the transposes of t+1, so the PE
            # never stalls on the (DVE) qT copy of its own group.
            def emit_transposes(t, split_copy=False):
                pt = psum_t.tile([P, 2, C, 2], FP8, tag="tr", name="pt")
                for kk in range(2):
                    k = 2 * t + kk
                    for ci in range(CT):
                        nc.tensor.transpose(
                            pt[:, kk, ci * P : (ci + 1) * P, 0],
                            q8[:, ci, k * P : (k + 1) * P],
                            id8[:],
                        )
                if split_copy:
                    nc.scalar.copy(
                        out=qT[:, 2 * t, :], in_=pt[:, 0, :, 0]
                    )
                    nc.vector.tensor_copy(
                        out=qT[:, 2 * t + 1, :], in_=pt[:, 1, :, 0]
                    )
                else:
                    nc.vector.tensor_copy(
                        out=qT[:, 2 * t : 2 * t + 2, :], in_=pt[:, :, :, 0]
                    )

            def emit_energy(t):
                for ci in range(CT):
                    nc.tensor.matmul(
                        e_ps[ci][:, ci * P :],
                        lhsT=qT[:, 2 * t : 2 * t + 2, ci * P : (ci + 1) * P],
                        rhs=qT[:, 2 * t : 2 * t + 2, ci * P :],
                        start=(t == 0),
                        stop=(t == KT // 2 - 1),
                        perf_mode=DR,
                    )

            for gg in range(4):
                base = gg * 1024
                if gg in (0, 3):
                    t0 = gg * 4
                    for ci in range(CT):
                        cast(ci, base, base + 512, "s" if ci < 2 else "v")
                    emit_transposes(t0, split_copy=(gg == 3))
                    if t0 > 0:
                        emit_energy(t0 - 1)
                    emit_transposes(t0 + 1, split_copy=(gg == 3))
                    for ci in range(CT):
                        cast(ci, base + 512, base + 1024, "s" if ci < 2 else "v")
                    emit_transposes(t0 + 2, split_copy=(gg == 3))
                    emit_energy(t0)
                    emit_transposes(t0 + 3, split_copy=(gg == 3))
                    emit_energy(t0 + 1)
                    emit_energy(t0 + 2)
                else:
                    for ci in range(CT):
                        cast(ci, base, base + 1024, "s")
                    for tt in range(4):
                        t = gg * 4 + tt
                        emit_transposes(t)
                        if t > 0:
                            emit_energy(t - 1)
            emit_energy(KT // 2 - 1)

            # ---- Phases B+C, pipelined per ci: mirror the lower-triangle
            # energy blocks this row needs (from the symmetric upper ones),
            # softmax, transpose the attention rows, att@q, fused
            # row-scale + x add, stream out.
            EXPQ = singles.tile([P, CT, C], FP8, tag="EXPQ")
            EXPT = singles.tile([P, CT, C], FP8, tag="EXPT")
            mcol = [singles.tile([P, 1], FP32, tag=f"m{ci}", name=f"m{ci}") for ci in range(CT)]
            zcol = [singles.tile([P, 1], FP32, tag=f"z{ci}", name=f"z{ci}") for ci in range(CT)]
            bias2 = [singles.tile([P, 1], FP32, tag=f"b{ci}", name=f"b{ci}") for ci in range(CT)]

            # All of phase B is emitted before any phase-C STT so the DVE
            # queue never buries a later row-block's min/mirror behind a
            # batch of output tiles (in-order engine queues).
            for ci in range(CT):
                # mirrors feeding row-block ci (none for ci == 0)
                for dj in range(ci):
                    low = mst.tile([P, P], FP32, tag="low", name="low")
                    nc.scalar.copy(
                        out=low[:], in_=e_ps[dj][:, ci * P : (ci + 1) * P]
                    )
                    nc.tensor.transpose(
                        e_ps[ci][:, dj * P : (dj + 1) * P], low[:], id32[:]
                    )
                nc.vector.tensor_reduce(
                    out=mcol[ci][:],
                    in_=e_ps[ci][:],
                    axis=mybir.AxisListType.X,
                    op=ALU.min,
                )
                nc.scalar.activation(
                    out=EXPQ[:, ci, :],
                    in_=e_ps[ci][:],
                    func=mybir.ActivationFunctionType.Exp,
                    bias=mcol[ci][:],
                    scale=-1.0,
                    accum_out=zcol[ci][:],
                )
                # Z >= 1 always (min element contributes exp(0)), no clamp.
                nc.vector.reciprocal(out=bias2[ci][:], in_=zcol[ci][:])
                nc.vector.tensor_mul(
                    out=bias2[ci][:], in0=bias2[ci][:], in1=gcol[:]
                )
                ptx = psum_t.tile([P, 2, C, 2], FP8, tag="tr", name="ptx")
                for dj in range(CT):
                    nc.tensor.transpose(
                        ptx[:, dj // 2, (dj % 2) * P : (dj % 2 + 1) * P, 0],
                        EXPQ[:, ci, dj * P : (dj + 1) * P],
                        id8[:],
                    )
                for a in range(2):
                    nc.scalar.copy(
                        out=EXPT[:, 2 * a : 2 * a + 2, ci * P : (ci + 1) * P],
                        in_=ptx[:, a, 0 : 2 * P, 0],
                    )

            # ---- Phase C: att@q, fused row-scale + x add, stream out.
            # Row-blocks 2-3 borrow the freed energy banks for a deeper
            # po pipeline.
            for ci in range(CT):
                for nh in range(2):
                    osb = stage.tile([P, 2048], FP32, tag="osb", name="osb")
                    for sub in range(4):
                        nj = nh * 4 + sub
                        pool = psum_o if ci < 2 else psum_e
                        po = pool.tile(
                            [P, C], FP32,
                            tag=("po" if ci < 2 else "acc"),
                            name="po",
                        )
                        for j in range(2):
                            nc.tensor.matmul(
                                po[:],
                                lhsT=EXPT[:, 2 * j : 2 * j + 2, ci * P : (ci + 1) * P],
                                rhs=q8[:, 2 * j : 2 * j + 2, nj * C : (nj + 1) * C],
                                start=(j == 0),
                                stop=(j == 1),
                                perf_mode=DR,
                            )
                        nc.vector.scalar_tensor_tensor(
                            out=osb[:, sub * C : (sub + 1) * C],
                            in0=po[:],
                            scalar=bias2[ci][:],
                            in1=xf[ci][:, nj * C : (nj + 1) * C],
                            op0=ALU.mult,
                            op1=ALU.add,
                        )
                        if sub % 2 == 1:
                            nc.sync.dma_start(
                                out=o_d[
                                    ci * P : (ci + 1) * P,
                                    (nj - 1) * C : (nj + 1) * C,
                                ],
                                in_=osb[:, (sub - 1) * C : (sub + 1) * C],
                            )
    _legalize_sync_waits(nc)
    return nc


def make_in_maps(x, gamma):
    x = np.ascontiguousarray(np.asarray(x, dtype=np.float32)).reshape(B, C, N)
    g = np.ascontiguousarray(np.asarray(gamma, dtype=np.float32)).reshape(1, 1)
    return [{"x": x[i], "gamma": g} for i in range(B)]


def kernel(x, y=None, gamma=None, **_ignored):
    from concourse.bass_utils import run_bass_kernel_spmd

    nc = build_nc()
    in_maps = make_in_maps(x, gamma)
    res = run_bass_kernel_spmd(nc, in_maps, list(range(B)))
    out = np.stack([np.asarray(res.results[i]["out"]) for i in range(B)])
    return out.reshape(B, C, 64, 64).astype(np.float32)
